# revision 25
# baseline (speedup 1.0000x reference)
"""CAM (channel attention module) Trainium2 kernel, v4.

Reference computation (per sample b):
    xf = x[b].reshape(C, N)
    energy = xf @ xf.T                      # [C, C]
    att = softmax(rowmin(energy) - energy)  # == softmax(-energy) rowwise
    out = (gamma * att + I) @ xf            # = gamma*(att@xf) + xf

Full shapes: x [128, 3, 16, 112, 112] f32, gamma [1] f32.
Data-parallel over batch: 16 samples per core on 8 NeuronCores.

v4 design (measured v1 330us -> v3 207us -> this):
- bf16 I/O end to end (rel err ~2e-3 << 2e-2 tolerance).
- QUAD packing: 4 samples share one 128-partition tile (32 partition
  rows each, 6272 free elems per row). Amortizes per-instruction fixed
  costs, quarters the weight-build count, halves PE matmul count.
- Gram: Act squares w/ accum_out; DVE fused STT (product+accum). The
  per-sample partition sums fall out of the chain's reduce matmul
  against a [P, 4] half-selector.
- Apply runs on the PE as scaled-identity matmuls accumulating over d:
      psum_c += (m_cd * I128) @ x_d
  where mI is built with per-partition TS scalars (mb varies by
  partition half), yielding the 4-sample block-diagonal automatically.
- PSUM->SBUF evacuation (fp32->bf16) split DVE/Act.
"""

import sys

sys.path.insert(0, "/opt/trn_rl_repo")

import numpy as np
import ml_dtypes

import concourse.bass as bass
import concourse.tile as tile
from concourse import mybir
from concourse.bass_utils import run_bass_kernel_spmd

B, C, T, H, W = 128, 3, 16, 112, 112
N = T * H * W                 # 200704
P = 128
NCORES = 8
S = B // NCORES               # 16 samples per core
PACK = 4                      # samples per quad tile
PP = P // PACK                # 32 partition rows per sample
FQ = N // PP                  # 6272 free elems per partition row
HQ = FQ // 2                  # 3136, half-quad for output streaming
NQ = S // PACK                # 4 quads per core

FP32 = mybir.dt.float32
BF16 = mybir.dt.bfloat16
AX = mybir.AxisListType
ALU = mybir.AluOpType
ACT = mybir.ActivationFunctionType

BF16NP = ml_dtypes.bfloat16

# per (c, half): two 1536-wide main psum segments + one 64-wide tail
SEG = 1536
TAIL = HQ - 2 * SEG           # 64
MM = 512                      # moving-dim chunk per matmul


def _bcast_last(ap, n):
    """[p, k] -> [p, k, n] with 0-stride last dim."""
    return bass.AP(tensor=ap.tensor, offset=ap.offset, ap=[*ap.ap, [0, n]])


def split_multi_waits(nc):
    """This container's walrus accepts only one sync-wait per instruction.
    Hoist extra waits onto single-wait NOPs on the same (in-order) queue."""
    n_split = 0
    for bb in nc.main_func.blocks:
        insts = list(bb.instructions)
        new = []
        for inst in insts:
            si = inst.sync_info
            waits = list(si.on_wait) if si is not None else []
            if len(waits) > 1:
                for i, w in enumerate(waits[:-1]):
                    nop = mybir.InstNoOp(
                        name=f"{inst.name}-wsplit{i}",
                        opcode="NoOp",
                        engine=inst.engine,
                        text_hint="wait_split",
                        bass_nofuse=True,
                        sync_info=mybir.SyncInfo(on_wait=[w], on_update=[]),
                    )
                    new.append(nop)
                    n_split += 1
                inst.sync_info = mybir.SyncInfo(
                    on_wait=[waits[-1]], on_update=list(si.on_update)
                )
            new.append(inst)
        if len(new) != len(insts):
            try:
                bb.instructions = new
            except Exception:
                del bb.instructions[:]
                bb.instructions.extend(new)
    return n_split


def build_kernel():
    from contextlib import ExitStack

    nc = bass.Bass("TRN2", target_bir_lowering=False, debug=False)

    x_d = nc.dram_tensor("x", [NQ, P, C * FQ], BF16, kind="ExternalInput")
    g_d = nc.dram_tensor("gamma", [PACK, 1], FP32, kind="ExternalInput")
    w2_d = nc.dram_tensor("w2g", [6, 9], FP32, kind="ExternalInput")
    i9_d = nc.dram_tensor("i9g", [PACK, 9], FP32, kind="ExternalInput")
    id_d = nc.dram_tensor("i128", [P, P], BF16, kind="ExternalInput")
    s4t_d = nc.dram_tensor("sel4t", [P, PACK], FP32, kind="ExternalInput")
    s4_d = nc.dram_tensor("sel4", [PACK, P], FP32, kind="ExternalInput")
    o_d = nc.dram_tensor("out", [NQ, P, C, FQ], BF16, kind="ExternalOutput")

    with tile.TileContext(nc) as tc, ExitStack() as ctx:
        consts = ctx.enter_context(tc.tile_pool(name="consts", bufs=1))
        in_pool = ctx.enter_context(tc.tile_pool(name="in", bufs=3))
        out_pool = ctx.enter_context(tc.tile_pool(name="outp", bufs=2))
        g_pool = ctx.enter_context(tc.tile_pool(name="gscr", bufs=1))
        mi_pool = ctx.enter_context(tc.tile_pool(name="mi", bufs=2))
        parts_pool = ctx.enter_context(tc.tile_pool(name="parts", bufs=2))
        mb_pool = ctx.enter_context(tc.tile_pool(name="mb", bufs=2))
        small = ctx.enter_context(tc.tile_pool(name="small", bufs=2))
        psum_ap = ctx.enter_context(tc.tile_pool(name="psap", bufs=2, space="PSUM"))
        psum_tl = ctx.enter_context(tc.tile_pool(name="pstl", bufs=1, space="PSUM"))
        psum_ch = ctx.enter_context(tc.tile_pool(name="psch", bufs=1, space="PSUM"))

        # ---- constants ----
        w2g = consts.tile([6, 9], FP32)
        nc.sync.dma_start(out=w2g, in_=w2_d.ap())
        i9g = consts.tile([PACK, 9], FP32)
        nc.sync.dma_start(out=i9g, in_=i9_d.ap())
        i128 = consts.tile([P, P], BF16)
        nc.sync.dma_start(out=i128, in_=id_d.ap())
        sel4t = consts.tile([P, PACK], FP32)
        nc.sync.dma_start(out=sel4t, in_=s4t_d.ap())
        sel4 = consts.tile([PACK, P], FP32)
        nc.sync.dma_start(out=sel4, in_=s4_d.ap())
        gamma_sb = consts.tile([PACK, 1], FP32)
        nc.sync.dma_start(out=gamma_sb, in_=g_d.ap())
        # engine-private garbage targets for accumulate-only passes
        garb_dve = consts.tile([P, FQ], BF16)
        garb_act = consts.tile([P, FQ], BF16)

        xin_tiles = {}
        parts_tiles = {}
        mb_tiles = {}
        mi_tiles = {}

        def emit_load(q):
            xin = in_pool.tile([P, C, FQ], BF16, tag="xin")
            nc.sync.dma_start(
                out=xin.rearrange("p c f -> p (c f)"), in_=x_d.ap()[q]
            )
            xin_tiles[q] = xin

        def emit_gram(q):
            xin = xin_tiles[q]
            parts = parts_pool.tile([P, 6], FP32, tag="parts")
            parts_tiles[q] = parts
            # Act: squares with accumulate -> parts[:, c]
            for c in range(C):
                nc.scalar.activation(
                    out=garb_act,
                    in_=xin[:, c, :],
                    func=ACT.Square,
                    accum_out=parts[:, c : c + 1],
                )
            # DVE: fused cross product + accumulate (STT w/ accum_out)
            for j, (a, b) in enumerate([(0, 1), (1, 2), (0, 2)]):
                nc.vector.scalar_tensor_tensor(
                    out=garb_dve,
                    in0=xin[:, a, :],
                    scalar=1.0,
                    in1=xin[:, b, :],
                    op0=ALU.mult,
                    op1=ALU.mult,
                    accum_out=parts[:, 3 + j : 4 + j],
                )

        def emit_chain(q):
            parts = parts_tiles[q]
            # one shared PSUM bank for the whole chain (disjoint regions)
            ch_ps = psum_ch.tile([P, 512], FP32, tag="chain")
            p14_ps = ch_ps[0:6, 0:PACK]
            e4_ps = ch_ps[0:PACK, 64 : 64 + 9]
            mb_ps = ch_ps[:, 192 : 192 + 9]
            # per-half partition sums: [6 dots, 4 halves]
            nc.tensor.matmul(out=p14_ps, lhsT=parts, rhs=sel4t)
            p14 = small.tile([6, PACK], FP32, tag="p14")
            nc.vector.tensor_copy(out=p14, in_=p14_ps)
            # gather into energies, one row per packed sample:
            # e4[h, j] = sum_k p14[k, h] * w2[k, j]
            nc.tensor.matmul(out=e4_ps, lhsT=p14, rhs=w2g)
            e4 = small.tile([PACK, 9], FP32, tag="e4")
            nc.vector.tensor_copy(out=e4, in_=e4_ps)
            e3 = e4.rearrange("p (c d) -> p c d", d=3)
            rmin = small.tile([PACK, 3], FP32, tag="rmin")
            nc.vector.tensor_reduce(out=rmin, in_=e3, axis=AX.X, op=ALU.min)
            z = small.tile([PACK, 9], FP32, tag="z")
            nc.vector.scalar_tensor_tensor(
                out=z.rearrange("p (c d) -> p c d", d=3),
                in0=e3, scalar=-1.0, in1=_bcast_last(rmin, 3),
                op0=ALU.mult, op1=ALU.add,
            )
            ex = small.tile([PACK, 9], FP32, tag="ex")
            nc.scalar.activation(out=ex, in_=z, func=ACT.Exp)
            ex3 = ex.rearrange("p (c d) -> p c d", d=3)
            sm = small.tile([PACK, 3], FP32, tag="sm")
            nc.vector.tensor_reduce(out=sm, in_=ex3, axis=AX.X, op=ALU.add)
            lnsm = small.tile([PACK, 3], FP32, tag="lnsm")
            nc.scalar.activation(out=lnsm, in_=sm, func=ACT.Ln)
            wv = small.tile([PACK, 9], FP32, tag="wv")
            nc.vector.scalar_tensor_tensor(
                out=wv.rearrange("p (c d) -> p c d", d=3),
                in0=z.rearrange("p (c d) -> p c d", d=3),
                scalar=1.0, in1=_bcast_last(lnsm, 3),
                op0=ALU.mult, op1=ALU.subtract,
            )
            att = small.tile([PACK, 9], FP32, tag="att")
            nc.scalar.activation(out=att, in_=wv, func=ACT.Exp)
            mflat = small.tile([PACK, 9], FP32, tag="mflat")
            nc.vector.scalar_tensor_tensor(
                out=mflat, in0=att, scalar=gamma_sb, in1=i9g,
                op0=ALU.mult, op1=ALU.add,
            )
            # broadcast per-half coefficients onto partitions
            nc.tensor.matmul(out=mb_ps, lhsT=sel4, rhs=mflat)
            mb = mb_pool.tile([P, 9], FP32, tag="mb")
            nc.scalar.copy(mb, mb_ps)
            mb_tiles[q] = mb

        def emit_mi(q):
            """Block-diagonal scaled-identity weights mI[3c+d]: per-
            partition TS scalar (mb varies by partition half)."""
            mb = mb_tiles[q]
            mis = []
            for j in range(9):
                mi = mi_pool.tile([P, P], BF16, tag=f"mi{j}")
                sc = mb[:, j : j + 1]
                if j % 3 == 0:
                    nc.scalar.mul(mi, i128, sc)
                else:
                    nc.vector.tensor_scalar(
                        out=mi, in0=i128, scalar1=sc, scalar2=None,
                        op0=ALU.mult,
                    )
                mis.append(mi)
            mi_tiles[q] = mis

        def emit_apply(q):
            """PE: psum_c += (m_cd I) @ x_d per half; evac DVE/Act."""
            xin = xin_tiles[q]
            mis = mi_tiles[q]
            ei = 0
            for half in range(2):
                hoff = half * HQ
                out_h = out_pool.tile([P, C, HQ], BF16, tag="outt")
                tail_ps = psum_tl.tile([P, C * TAIL], FP32, tag="tail")
                for c in range(C):
                    seg0_ps = psum_ap.tile([P, SEG], FP32, tag="ap")
                    seg1_ps = psum_ap.tile([P, SEG], FP32, tag="ap")
                    seg_ps = [seg0_ps, seg1_ps]
                    for d in range(C):
                        st, sp = (d == 0), (d == 2)
                        for ch in range(6):
                            nc.tensor.matmul(
                                out=seg_ps[ch // 3][:, (ch % 3) * MM : (ch % 3 + 1) * MM],
                                lhsT=mis[3 * c + d],
                                rhs=xin[:, d, hoff + ch * MM : hoff + (ch + 1) * MM],
                                start=st, stop=sp,
                            )
                        nc.tensor.matmul(
                            out=tail_ps[:, TAIL * c : TAIL * (c + 1)],
                            lhsT=mis[3 * c + d],
                            rhs=xin[:, d, hoff + 2 * SEG : hoff + HQ],
                            start=st, stop=sp,
                        )
                    # evacuate the two main segments (alternate DVE/Act)
                    for sgi in range(2):
                        dst = out_h[:, c, sgi * SEG : (sgi + 1) * SEG]
                        if ei % 2 == 0:
                            nc.vector.tensor_copy(out=dst, in_=seg_ps[sgi])
                        else:
                            nc.scalar.copy(dst, seg_ps[sgi])
                        ei += 1
                # tail: one strided copy [P, 3, TAIL]
                tl = tail_ps.rearrange("p (c t) -> p c t", t=TAIL)
                dst = out_h[:, :, 2 * SEG : HQ]
                if half % 2 == 0:
                    nc.vector.tensor_copy(out=dst, in_=tl)
                else:
                    nc.scalar.copy(dst, tl)
                nc.sync.dma_start(
                    out=o_d.ap()[q][:, :, hoff : hoff + HQ], in_=out_h
                )
            del xin_tiles[q], mi_tiles[q]

        # ---- schedule ----
        emit_load(0)
        emit_load(1)
        emit_gram(0)
        emit_chain(0)
        for q in range(NQ):
            if q + 2 < NQ:
                emit_load(q + 2)
            if q + 1 < NQ:
                emit_gram(q + 1)
            emit_mi(q)
            emit_apply(q)
            if q + 1 < NQ:
                emit_chain(q + 1)

    split_multi_waits(nc)
    return nc


def const_inputs():
    # parts row order per quad: [x0^2, x1^2, x2^2, x0x1, x1x2, x0x2]
    # energies e[3a+b] = sum_n x_a x_b
    w2 = np.zeros((6, 9), np.float32)
    for c in range(3):
        w2[c, 4 * c] = 1.0
    for j, (a, b) in enumerate([(0, 1), (1, 2), (0, 2)]):
        w2[3 + j, 3 * a + b] = 1.0
        w2[3 + j, 3 * b + a] = 1.0
    i9g = np.tile(np.eye(3, dtype=np.float32).reshape(1, 9), (PACK, 1))
    i128 = np.eye(P, dtype=BF16NP)
    sel4t = np.zeros((P, PACK), np.float32)
    for h in range(PACK):
        sel4t[h * PP : (h + 1) * PP, h] = 1.0
    sel4 = np.ascontiguousarray(sel4t.T)
    return {"w2g": w2, "i9g": i9g, "i128": i128, "sel4t": sel4t, "sel4": sel4}


_NC_CACHE = {}


def _get_nc():
    if "v4" not in _NC_CACHE:
        _NC_CACHE["v4"] = build_kernel()
    return _NC_CACHE["v4"]


def _prep_inputs(x, gamma):
    # [B, C, T, H, W] -> per-core quads [NQ, P, C*FQ] bf16, where
    # partition p of quad q holds sample 4q + p//32, rows n = (p%32)*FQ..
    xs = np.ascontiguousarray(x).reshape(NCORES, NQ, PACK, C, PP, FQ)
    xs = np.transpose(xs, (0, 1, 2, 4, 3, 5))       # [8, NQ, PACK, PP, C, FQ]
    xs = xs.astype(BF16NP).reshape(NCORES, NQ, P, C * FQ)
    g = np.full((PACK, 1), np.float32(np.asarray(gamma).reshape(-1)[0]))
    cns = const_inputs()
    return [{"x": xs[i], "gamma": g, **cns} for i in range(NCORES)]


def _assemble_out(res):
    out = np.stack([np.asarray(res.results[i]["out"]) for i in range(NCORES)])
    out = out.reshape(NCORES, NQ, PACK, PP, C, FQ).astype(np.float32)
    out = np.transpose(out, (0, 1, 2, 4, 3, 5))     # [8, NQ, PACK, C, PP, FQ]
    return np.ascontiguousarray(out).reshape(B, C, T, H, W)


def kernel(x: np.ndarray, gamma: np.ndarray) -> np.ndarray:
    assert x.shape == (B, C, T, H, W) and x.dtype == np.float32
    nc = _get_nc()
    in_maps = _prep_inputs(x, gamma)
    res = run_bass_kernel_spmd(nc, in_maps, core_ids=list(range(NCORES)))
    return _assemble_out(res)


def _install_ntff_hook():
    """The image's antenv lacks axon_hooks; synthesize it so
    run_bass_kernel_spmd(trace=True) can capture NTFF profiles."""
    import types

    try:
        from antenv.axon_hooks import get_axon_ntff_profile_hook  # noqa: F401

        return True
    except ImportError:
        pass
    try:
        import antenv

        mod = types.ModuleType("antenv.axon_hooks")
        _state = {"hook": None}

        def set_axon_ntff_profile_hook(h):
            _state["hook"] = h

        def get_axon_ntff_profile_hook():
            return _state["hook"]

        mod.set_axon_ntff_profile_hook = set_axon_ntff_profile_hook
        mod.get_axon_ntff_profile_hook = get_axon_ntff_profile_hook
        sys.modules["antenv.axon_hooks"] = mod
        antenv.axon_hooks = mod

        sys.path.insert(0, "/root/.axon_site")
        from trn_agent_boot.trn_boot import _ntff_profile_via_ctypes

        hook = _ntff_profile_via_ctypes("/opt/axon/libaxon_pjrt.so")
        if hook is None:
            return False
        set_axon_ntff_profile_hook(hook)
        return True
    except Exception as e:  # pragma: no cover
        print("ntff hook install failed:", e)
        return False


def profile_once(inputs):
    """Run with NTFF tracing; returns max per-core exec_time_ns."""
    _install_ntff_hook()
    nc = _get_nc()
    in_maps = _prep_inputs(np.asarray(inputs["x"]), inputs["gamma"])
    res = run_bass_kernel_spmd(
        nc, in_maps, core_ids=list(range(NCORES)), trace=True
    )
    print("profile_json:", res.profile_json)
    print("exec_time_ns:", res.exec_time_ns, "mean:", res.mean_exec_time_ns)
    return res.exec_time_ns


if __name__ == "__main__":
    x = np.random.randn(B, C, T, H, W).astype(np.float32)
    gamma = np.zeros((1,), np.float32)
    y = kernel(x, gamma)
    print("ok", y.shape, float(np.abs(y - x.astype(BF16NP).astype(np.float32)).max()))


# revision 32
# speedup vs baseline: 1.0129x; 1.0129x over previous
"""CAM (channel attention module) Trainium2 kernel, v4.

Reference computation (per sample b):
    xf = x[b].reshape(C, N)
    energy = xf @ xf.T                      # [C, C]
    att = softmax(rowmin(energy) - energy)  # == softmax(-energy) rowwise
    out = (gamma * att + I) @ xf            # = gamma*(att@xf) + xf

Full shapes: x [128, 3, 16, 112, 112] f32, gamma [1] f32.
Data-parallel over batch: 16 samples per core on 8 NeuronCores.

v4 design (measured v1 330us -> v3 207us -> this):
- bf16 I/O end to end (rel err ~2e-3 << 2e-2 tolerance).
- QUAD packing: 4 samples share one 128-partition tile (32 partition
  rows each, 6272 free elems per row). Amortizes per-instruction fixed
  costs, quarters the weight-build count, halves PE matmul count.
- Gram: Act squares w/ accum_out; DVE fused STT (product+accum). The
  per-sample partition sums fall out of the chain's reduce matmul
  against a [P, 4] half-selector.
- Apply runs on the PE as scaled-identity matmuls accumulating over d:
      psum_c += (m_cd * I128) @ x_d
  where mI is built with per-partition TS scalars (mb varies by
  partition half), yielding the 4-sample block-diagonal automatically.
- PSUM->SBUF evacuation (fp32->bf16) split DVE/Act.
"""

import sys

sys.path.insert(0, "/opt/trn_rl_repo")

import numpy as np
import ml_dtypes

import concourse.bass as bass
import concourse.tile as tile
from concourse import mybir
from concourse.bass_utils import run_bass_kernel_spmd

B, C, T, H, W = 128, 3, 16, 112, 112
N = T * H * W                 # 200704
P = 128
NCORES = 8
S = B // NCORES               # 16 samples per core
PACK = 4                      # samples per quad tile
PP = P // PACK                # 32 partition rows per sample
FQ = N // PP                  # 6272 free elems per partition row
HQ = FQ // 2                  # 3136, half-quad for output streaming
NQ = S // PACK                # 4 quads per core

FP32 = mybir.dt.float32
BF16 = mybir.dt.bfloat16
AX = mybir.AxisListType
ALU = mybir.AluOpType
ACT = mybir.ActivationFunctionType

BF16NP = ml_dtypes.bfloat16

# per (c, half): two 1536-wide main psum segments + one 64-wide tail
SEG = 1536
TAIL = HQ - 2 * SEG           # 64
MM = 512                      # moving-dim chunk per matmul


def _bcast_last(ap, n):
    """[p, k] -> [p, k, n] with 0-stride last dim."""
    return bass.AP(tensor=ap.tensor, offset=ap.offset, ap=[*ap.ap, [0, n]])


def _bcast_col(ap, n):
    """[p, 1] -> [p, n] with 0-stride free dim."""
    return bass.AP(tensor=ap.tensor, offset=ap.offset, ap=[ap.ap[0], [0, n]])


def split_multi_waits(nc):
    """This container's walrus accepts only one sync-wait per instruction.
    Hoist extra waits onto single-wait NOPs on the same (in-order) queue."""
    n_split = 0
    for bb in nc.main_func.blocks:
        insts = list(bb.instructions)
        new = []
        for inst in insts:
            si = inst.sync_info
            waits = list(si.on_wait) if si is not None else []
            if len(waits) > 1:
                for i, w in enumerate(waits[:-1]):
                    nop = mybir.InstNoOp(
                        name=f"{inst.name}-wsplit{i}",
                        opcode="NoOp",
                        engine=inst.engine,
                        text_hint="wait_split",
                        bass_nofuse=True,
                        sync_info=mybir.SyncInfo(on_wait=[w], on_update=[]),
                    )
                    new.append(nop)
                    n_split += 1
                inst.sync_info = mybir.SyncInfo(
                    on_wait=[waits[-1]], on_update=list(si.on_update)
                )
            new.append(inst)
        if len(new) != len(insts):
            try:
                bb.instructions = new
            except Exception:
                del bb.instructions[:]
                bb.instructions.extend(new)
    return n_split


def build_kernel():
    from contextlib import ExitStack

    nc = bass.Bass("TRN2", target_bir_lowering=False, debug=False)

    x_d = nc.dram_tensor("x", [NQ, P, C * FQ], BF16, kind="ExternalInput")
    g_d = nc.dram_tensor("gamma", [PACK, 1], FP32, kind="ExternalInput")
    w2_d = nc.dram_tensor("w2g", [6, 9], FP32, kind="ExternalInput")
    i9_d = nc.dram_tensor("i9g", [PACK, 9], FP32, kind="ExternalInput")
    id_d = nc.dram_tensor("i128", [P, P], BF16, kind="ExternalInput")
    s4t_d = nc.dram_tensor("sel4t", [P, PACK], FP32, kind="ExternalInput")
    s4_d = nc.dram_tensor("sel4", [PACK, P], FP32, kind="ExternalInput")
    o_d = nc.dram_tensor("out", [NQ, P, C, FQ], BF16, kind="ExternalOutput")

    with tile.TileContext(nc) as tc, ExitStack() as ctx:
        consts = ctx.enter_context(tc.tile_pool(name="consts", bufs=1))
        in_pool = ctx.enter_context(tc.tile_pool(name="in", bufs=3))
        out_pool = ctx.enter_context(tc.tile_pool(name="outp", bufs=2))
        g_pool = ctx.enter_context(tc.tile_pool(name="gscr", bufs=1))
        mi_pool = ctx.enter_context(tc.tile_pool(name="mi", bufs=2))
        parts_pool = ctx.enter_context(tc.tile_pool(name="parts", bufs=2))
        mb_pool = ctx.enter_context(tc.tile_pool(name="mb", bufs=2))
        small = ctx.enter_context(tc.tile_pool(name="small", bufs=2))
        psum_ap = ctx.enter_context(tc.tile_pool(name="psap", bufs=2, space="PSUM"))
        psum_tl = ctx.enter_context(tc.tile_pool(name="pstl", bufs=1, space="PSUM"))
        psum_ch = ctx.enter_context(tc.tile_pool(name="psch", bufs=1, space="PSUM"))

        # ---- constants ----
        w2g = consts.tile([6, 9], FP32)
        nc.sync.dma_start(out=w2g, in_=w2_d.ap())
        i9g = consts.tile([PACK, 9], FP32)
        nc.sync.dma_start(out=i9g, in_=i9_d.ap())
        i128 = consts.tile([P, P], BF16)
        nc.sync.dma_start(out=i128, in_=id_d.ap())
        sel4t = consts.tile([P, PACK], FP32)
        nc.sync.dma_start(out=sel4t, in_=s4t_d.ap())
        sel4 = consts.tile([PACK, P], FP32)
        nc.sync.dma_start(out=sel4, in_=s4_d.ap())
        gamma_sb = consts.tile([PACK, 1], FP32)
        nc.sync.dma_start(out=gamma_sb, in_=g_d.ap())
        # engine-private garbage targets for accumulate-only passes
        garb_dve = consts.tile([P, FQ], BF16)
        garb_act = consts.tile([P, FQ], BF16)

        xin_tiles = {}
        parts_tiles = {}
        mb_tiles = {}
        mi_tiles = {}

        def emit_load(q):
            xin = in_pool.tile([P, C, FQ], BF16, tag="xin")
            nc.sync.dma_start(
                out=xin.rearrange("p c f -> p (c f)"), in_=x_d.ap()[q]
            )
            xin_tiles[q] = xin

        def gram_ops(q):
            """Six deferred gram ops (3 Act squares, 3 DVE fused STT dots),
            interleaved Act/DVE so either queue can make progress."""
            xin = xin_tiles[q]
            parts = parts_pool.tile([P, 6], FP32, tag="parts")
            parts_tiles[q] = parts

            def mk_sq(c):
                def op():
                    nc.scalar.activation(
                        out=garb_act,
                        in_=xin[:, c, :],
                        func=ACT.Square,
                        accum_out=parts[:, c : c + 1],
                    )
                return op

            def mk_cross(j, a, b):
                def op():
                    nc.vector.scalar_tensor_tensor(
                        out=garb_dve,
                        in0=xin[:, a, :],
                        scalar=1.0,
                        in1=xin[:, b, :],
                        op0=ALU.mult,
                        op1=ALU.mult,
                        accum_out=parts[:, 3 + j : 4 + j],
                    )
                return op

            crosses = [mk_cross(j, a, b) for j, (a, b) in
                       enumerate([(0, 1), (1, 2), (0, 2)])]
            sqs = [mk_sq(c) for c in range(C)]
            return [op for pair in zip(sqs, crosses) for op in pair]

        def emit_gram(q):
            for op in gram_ops(q):
                op()

        def emit_chain(q):
            parts = parts_tiles[q]
            # one shared PSUM bank for the whole chain (disjoint regions)
            ch_ps = psum_ch.tile([P, 512], FP32, tag="chain")
            p14_ps = ch_ps[0:6, 0:PACK]
            e4_ps = ch_ps[0:PACK, 64 : 64 + 9]
            mb_ps = ch_ps[:, 192 : 192 + 9]
            # per-half partition sums: [6 dots, 4 halves]
            nc.tensor.matmul(out=p14_ps, lhsT=parts, rhs=sel4t)
            p14 = small.tile([6, PACK], FP32, tag="p14")
            nc.vector.tensor_copy(out=p14, in_=p14_ps)
            # gather into energies, one row per packed sample:
            # e4[h, j] = sum_k p14[k, h] * w2[k, j]
            nc.tensor.matmul(out=e4_ps, lhsT=p14, rhs=w2g)
            e4 = small.tile([PACK, 9], FP32, tag="e4")
            nc.vector.tensor_copy(out=e4, in_=e4_ps)
            e3 = e4.rearrange("p (c d) -> p c d", d=3)
            rmin = small.tile([PACK, 3], FP32, tag="rmin")
            nc.vector.tensor_reduce(out=rmin, in_=e3, axis=AX.X, op=ALU.min)
            # z = rmin - e  (cheap TT; STT has a ~1.2us fixed cost)
            z = small.tile([PACK, 9], FP32, tag="z")
            nc.vector.tensor_tensor(
                out=z.rearrange("p (c d) -> p c d", d=3),
                in0=_bcast_last(rmin, 3), in1=e3, op=ALU.subtract,
            )
            ex = small.tile([PACK, 9], FP32, tag="ex")
            nc.scalar.activation(out=ex, in_=z, func=ACT.Exp)
            ex3 = ex.rearrange("p (c d) -> p c d", d=3)
            sm = small.tile([PACK, 3], FP32, tag="sm")
            nc.vector.tensor_reduce(out=sm, in_=ex3, axis=AX.X, op=ALU.add)
            rsm = small.tile([PACK, 3], FP32, tag="rsm")
            nc.vector.reciprocal(out=rsm, in_=sm)
            att = small.tile([PACK, 9], FP32, tag="att")
            nc.vector.tensor_tensor(
                out=att.rearrange("p (c d) -> p c d", d=3),
                in0=ex3, in1=_bcast_last(rsm, 3), op=ALU.mult,
            )
            # mflat = gamma * att + I
            gat = small.tile([PACK, 9], FP32, tag="gat")
            nc.vector.tensor_tensor(
                out=gat, in0=att, in1=_bcast_col(gamma_sb, 9), op=ALU.mult,
            )
            mflat = small.tile([PACK, 9], FP32, tag="mflat")
            nc.vector.tensor_tensor(out=mflat, in0=gat, in1=i9g, op=ALU.add)
            # broadcast per-half coefficients onto partitions
            nc.tensor.matmul(out=mb_ps, lhsT=sel4, rhs=mflat)
            mb = mb_pool.tile([P, 9], FP32, tag="mb")
            nc.scalar.copy(mb, mb_ps)
            mb_tiles[q] = mb

        def emit_mi(q):
            """Block-diagonal scaled-identity weights mI[3c+d]: per-
            partition TS scalar (mb varies by partition half)."""
            mb = mb_tiles[q]
            mis = []
            for j in range(9):
                mi = mi_pool.tile([P, P], BF16, tag=f"mi{j}")
                sc = mb[:, j : j + 1]
                if j % 3 == 0:
                    nc.scalar.mul(mi, i128, sc)
                else:
                    nc.vector.tensor_scalar(
                        out=mi, in0=i128, scalar1=sc, scalar2=None,
                        op0=ALU.mult,
                    )
                mis.append(mi)
            mi_tiles[q] = mis

        def emit_apply(q, pending=()):
            """PE: psum_c += (m_cd I) @ x_d per half; evac DVE/Act.
            One deferred gram(q+1) op is emitted per (half, c) block so
            DVE/Act stay fed while the PE fills the next psum."""
            pending = list(pending)
            xin = xin_tiles[q]
            mis = mi_tiles[q]
            ei = 0
            for half in range(2):
                hoff = half * HQ
                out_h = out_pool.tile([P, C, HQ], BF16, tag="outt")
                tail_ps = psum_tl.tile([P, C * TAIL], FP32, tag="tail")
                for c in range(C):
                    seg0_ps = psum_ap.tile([P, SEG], FP32, tag="ap")
                    seg1_ps = psum_ap.tile([P, SEG], FP32, tag="ap")
                    seg_ps = [seg0_ps, seg1_ps]
                    for d in range(C):
                        st, sp = (d == 0), (d == 2)
                        for ch in range(6):
                            nc.tensor.matmul(
                                out=seg_ps[ch // 3][:, (ch % 3) * MM : (ch % 3 + 1) * MM],
                                lhsT=mis[3 * c + d],
                                rhs=xin[:, d, hoff + ch * MM : hoff + (ch + 1) * MM],
                                start=st, stop=sp,
                            )
                        nc.tensor.matmul(
                            out=tail_ps[:, TAIL * c : TAIL * (c + 1)],
                            lhsT=mis[3 * c + d],
                            rhs=xin[:, d, hoff + 2 * SEG : hoff + HQ],
                            start=st, stop=sp,
                        )
                    # one gram(q+1) op ahead of this block's evacuations
                    if pending:
                        pending.pop(0)()
                    # evacuate the two main segments (alternate DVE/Act)
                    for sgi in range(2):
                        dst = out_h[:, c, sgi * SEG : (sgi + 1) * SEG]
                        if ei % 2 == 0:
                            nc.vector.tensor_copy(out=dst, in_=seg_ps[sgi])
                        else:
                            nc.scalar.copy(dst, seg_ps[sgi])
                        ei += 1
                # tail: one strided copy [P, 3, TAIL]
                tl = tail_ps.rearrange("p (c t) -> p c t", t=TAIL)
                dst = out_h[:, :, 2 * SEG : HQ]
                if half % 2 == 0:
                    nc.vector.tensor_copy(out=dst, in_=tl)
                else:
                    nc.scalar.copy(dst, tl)
                nc.sync.dma_start(
                    out=o_d.ap()[q][:, :, hoff : hoff + HQ], in_=out_h
                )
            del xin_tiles[q], mi_tiles[q]

        # ---- schedule ----
        emit_load(0)
        emit_load(1)
        emit_gram(0)
        emit_chain(0)
        for q in range(NQ):
            if q + 2 < NQ:
                emit_load(q + 2)
            emit_mi(q)
            pend = gram_ops(q + 1) if q + 1 < NQ else ()
            emit_apply(q, pend)
            if q + 1 < NQ:
                emit_chain(q + 1)

    split_multi_waits(nc)
    return nc


def const_inputs():
    # parts row order per quad: [x0^2, x1^2, x2^2, x0x1, x1x2, x0x2]
    # energies e[3a+b] = sum_n x_a x_b
    w2 = np.zeros((6, 9), np.float32)
    for c in range(3):
        w2[c, 4 * c] = 1.0
    for j, (a, b) in enumerate([(0, 1), (1, 2), (0, 2)]):
        w2[3 + j, 3 * a + b] = 1.0
        w2[3 + j, 3 * b + a] = 1.0
    i9g = np.tile(np.eye(3, dtype=np.float32).reshape(1, 9), (PACK, 1))
    i128 = np.eye(P, dtype=BF16NP)
    sel4t = np.zeros((P, PACK), np.float32)
    for h in range(PACK):
        sel4t[h * PP : (h + 1) * PP, h] = 1.0
    sel4 = np.ascontiguousarray(sel4t.T)
    return {"w2g": w2, "i9g": i9g, "i128": i128, "sel4t": sel4t, "sel4": sel4}


_NC_CACHE = {}


def _get_nc():
    if "v4" not in _NC_CACHE:
        _NC_CACHE["v4"] = build_kernel()
    return _NC_CACHE["v4"]


def _prep_inputs(x, gamma):
    # [B, C, T, H, W] -> per-core quads [NQ, P, C*FQ] bf16, where
    # partition p of quad q holds sample 4q + p//32, rows n = (p%32)*FQ..
    xs = np.ascontiguousarray(x).reshape(NCORES, NQ, PACK, C, PP, FQ)
    xs = np.transpose(xs, (0, 1, 2, 4, 3, 5))       # [8, NQ, PACK, PP, C, FQ]
    xs = xs.astype(BF16NP).reshape(NCORES, NQ, P, C * FQ)
    g = np.full((PACK, 1), np.float32(np.asarray(gamma).reshape(-1)[0]))
    cns = const_inputs()
    return [{"x": xs[i], "gamma": g, **cns} for i in range(NCORES)]


def _assemble_out(res):
    out = np.stack([np.asarray(res.results[i]["out"]) for i in range(NCORES)])
    out = out.reshape(NCORES, NQ, PACK, PP, C, FQ).astype(np.float32)
    out = np.transpose(out, (0, 1, 2, 4, 3, 5))     # [8, NQ, PACK, C, PP, FQ]
    return np.ascontiguousarray(out).reshape(B, C, T, H, W)


def kernel(x: np.ndarray, gamma: np.ndarray) -> np.ndarray:
    assert x.shape == (B, C, T, H, W) and x.dtype == np.float32
    nc = _get_nc()
    in_maps = _prep_inputs(x, gamma)
    res = run_bass_kernel_spmd(nc, in_maps, core_ids=list(range(NCORES)))
    return _assemble_out(res)


def _install_ntff_hook():
    """The image's antenv lacks axon_hooks; synthesize it so
    run_bass_kernel_spmd(trace=True) can capture NTFF profiles."""
    import types

    try:
        from antenv.axon_hooks import get_axon_ntff_profile_hook  # noqa: F401

        return True
    except ImportError:
        pass
    try:
        import antenv

        mod = types.ModuleType("antenv.axon_hooks")
        _state = {"hook": None}

        def set_axon_ntff_profile_hook(h):
            _state["hook"] = h

        def get_axon_ntff_profile_hook():
            return _state["hook"]

        mod.set_axon_ntff_profile_hook = set_axon_ntff_profile_hook
        mod.get_axon_ntff_profile_hook = get_axon_ntff_profile_hook
        sys.modules["antenv.axon_hooks"] = mod
        antenv.axon_hooks = mod

        sys.path.insert(0, "/root/.axon_site")
        from trn_agent_boot.trn_boot import _ntff_profile_via_ctypes

        hook = _ntff_profile_via_ctypes("/opt/axon/libaxon_pjrt.so")
        if hook is None:
            return False
        set_axon_ntff_profile_hook(hook)
        return True
    except Exception as e:  # pragma: no cover
        print("ntff hook install failed:", e)
        return False


def profile_once(inputs):
    """Run with NTFF tracing; returns max per-core exec_time_ns."""
    _install_ntff_hook()
    nc = _get_nc()
    in_maps = _prep_inputs(np.asarray(inputs["x"]), inputs["gamma"])
    res = run_bass_kernel_spmd(
        nc, in_maps, core_ids=list(range(NCORES)), trace=True
    )
    print("profile_json:", res.profile_json)
    print("exec_time_ns:", res.exec_time_ns, "mean:", res.mean_exec_time_ns)
    return res.exec_time_ns


if __name__ == "__main__":
    x = np.random.randn(B, C, T, H, W).astype(np.float32)
    gamma = np.zeros((1,), np.float32)
    y = kernel(x, gamma)
    print("ok", y.shape, float(np.abs(y - x.astype(BF16NP).astype(np.float32)).max()))
